# revision 70
# baseline (speedup 1.0000x reference)
"""Multi-head causal self-attention on 8 TRN2 NeuronCores.

Problem (hardcoded): x[2,2048,1024] f32, Q/K/V/O [1024,1024] f32, 16 heads,
Dh=64, causal softmax, out = attn(x) @ O.T  -> [2,2048,1024] f32.

Sharding: core c handles batch b=c//4 and head group g=c%4 (4 heads each).
Each core computes a partial output (its heads' contribution through the O
projection); the host gather sums the 4 partials per batch (the all-reduce
of the hint, performed at unshard time).

Device algorithm per core (heads h=0..3):
  Phase 1: projections with fp32r matmuls (1 cyc/row at N>=256):
      qa[h][0:64, s] = (Q_h/8) @ x.T   per-head [65, S] tiles; row 64 later
                       receives -rowmax (written by DMA after the A-pass)
      ka[h][0:64, s] = K_h @ x.T       row 64 = ones (set once via DMA)
      v[s,d] bf16 (+ ones column for free softmax denominators)
      PSUM -> SBUF copies ride the DMA engines (frees ACT/DVE).
  Phase 2 per head:
      A-pass: scores[q,k] fp32r -> causal masked row max via DVE (negated)
      negmax [128,16] -PE transpose-> row [1,2048] -DMA-> qa[h] row 64
      T-pass: scores_T[k,q] with K=65 contraction (the ones row of ka times
              the -max row of qa subtracts the row max inside the same
              matmul -- no extra PE cost, cost model charges N only)
              diag causal mask add, then ACT exp -> PT bf16 (k-major)
      av: out[q,d]+denominator via PT.T @ [v|1], normalize by 1/l (DVE)
  Phase 3: ho[s,hd] -PE transpose-> hoT, out_partial = hoT.T @ O_cols.T (bf16)
"""
import numpy as np

import concourse.bass as bass
import concourse.tile as tile
from concourse import bacc, mybir
from concourse.bass_utils import run_bass_kernel_spmd
from concourse.masks import make_identity

F32 = mybir.dt.float32
F32R = mybir.dt.float32r
BF16 = mybir.dt.bfloat16

B, S, D, H = 2, 2048, 1024, 16
DH = 64          # head dim
HPC = 4          # heads per core
NB = S // 128    # 16 q/k blocks
FT = D // 128    # 8 f-tiles
NEG = -3.0e38

# PT column offsets: head-local P^T storage, block j spans q-cols [j*128, S)
PT_OFF = [0] * (NB + 1)
for _j in range(NB):
    PT_OFF[_j + 1] = PT_OFF[_j] + (S - _j * 128)
PT_COLS = PT_OFF[NB]  # 17408


def chunks_ge256(w):
    """Split a multiple-of-128 width into matmul chunks <=512, avoiding
    <256-wide chunks (fp32r moving runs 4 cyc/row below N=256). Only valid
    when each chunk lands in its own PSUM tile (A-pass)."""
    out = []
    while w:
        if w <= 512:
            out.append(w)
            break
        if w == 640:
            out += [384, 256]
            break
        out.append(512)
        w -= 512
    return out


def chunks_aligned(w):
    """512-aligned chunks for matmuls sharing one multi-bank PSUM tile:
    a single matmul output must not cross a PSUM bank (512 f32) boundary."""
    out = []
    while w:
        c = min(512, w)
        out.append(c)
        w -= c
    return out


def build_nc():
    nc = bacc.Bacc(None, target_bir_lowering=False, debug=False)

    # f32r in DRAM: bit-identical to the f32 host arrays, lets the loads go
    # cast-free on the HWDGE (sync) queue in parallel with the Pool queue
    xt_d = nc.dram_tensor("xt", [D, S], F32R, kind="ExternalInput")
    qt_d = nc.dram_tensor("qt", [D, 256], F32R, kind="ExternalInput")
    kt_d = nc.dram_tensor("kt", [D, 256], F32R, kind="ExternalInput")
    vt_d = nc.dram_tensor("vt", [D, 256], F32R, kind="ExternalInput")
    ot_d = nc.dram_tensor("ot", [256, D], F32, kind="ExternalInput")
    tri_d = nc.dram_tensor("tri", [128, 128], F32, kind="ExternalInput")
    rsh_d = nc.dram_tensor("rsh", [128, 128], F32, kind="ExternalInput")
    # bf16 output: halves the store traffic; the host gather sums partials
    # in f32 (bf16 rounding is ~0.4% relative, well inside tolerance)
    out_d = nc.dram_tensor("out", [S, D], BF16, kind="ExternalOutput")

    with tile.TileContext(nc) as tc:
        with (
            tc.tile_pool(name="singles", bufs=1) as singles,
            tc.tile_pool(name="mid", bufs=1) as mid,
        ):
            # whole-kernel constants / tensors
            ot_sb = singles.tile([128, 2, D], BF16)
            v_sb = [singles.tile([128, HPC, 65], BF16, name=f"v{j}")
                    for j in range(NB)]
            ho_sb = [singles.tile([128, 256], BF16, name=f"ho{i}")
                     for i in range(NB)]
            tri_sb = singles.tile([128, 128], BF16)
            rsh_sb = singles.tile([128, 128], BF16)
            ones_f32 = singles.tile([128, 128], F32)
            identf = singles.tile([128, 128], F32)
            identb = singles.tile([128, 128], BF16)

            # per-head augmented projections: rows 0..63 data, row 64 aug
            qa = [mid.tile([65, S], F32R, name=f"qa{h}") for h in range(HPC)]
            ka = [mid.tile([65, S], F32R, name=f"ka{h}") for h in range(HPC)]

            # ---------------- Phase 1: DMAs ----------------
            ph2_cm = tc.tile_pool(name="ph2", bufs=2)
            ph2 = ph2_cm.__enter__()
            psA_cm = tc.tile_pool(name="psA", bufs=2, space="PSUM")
            psA = psA_cm.__enter__()
            ph1_cm = tc.tile_pool(name="ph1", bufs=1)
            pp_cm = tc.tile_pool(name="pp", bufs=6, space="PSUM")
            ph1, pp = ph1_cm.__enter__(), pp_cm.__enter__()

            xt_sb = [ph1.tile([128, S], F32R, name=f"xt_sb{t}")
                     for t in range(FT)]
            qtw = ph1.tile([128, FT, 256], F32R)
            ktw = ph1.tile([128, FT, 256], F32R)
            vtw = ph1.tile([128, FT, 256], F32R)

            # column-group-major xt load: proj0 group g and the A0/A1 max
            # blocks needing cols < (g+1)*512 can run while later groups
            # still stream in; casting loads stay on the Pool queue
            qtv = qt_d[:].rearrange("(t p) m -> p t m", p=128)
            ktv = kt_d[:].rearrange("(t p) m -> p t m", p=128)
            nc.gpsimd.dma_start(tri_sb[:], tri_d[:])
            nc.gpsimd.dma_start(rsh_sb[:], rsh_d[:])
            # only pair-0's weight slices (one strided DMA each) ride
            # ahead of the group-0 xt quarters; pair-1 follows group 1
            nc.sync.dma_start(qtw[:, :, 0:128], qtv[:, :, 0:128])
            nc.sync.dma_start(ktw[:, :, 0:128], ktv[:, :, 0:128])
            for t in range(FT):
                nc.sync.dma_start(
                    xt_sb[t][:, 0:512],
                    xt_d[t * 128:(t + 1) * 128, 0:512])
            for g in range(1, 4):
                for t in range(FT):
                    nc.sync.dma_start(
                        xt_sb[t][:, g * 512:(g + 1) * 512],
                        xt_d[t * 128:(t + 1) * 128, g * 512:(g + 1) * 512])
                if g == 1:
                    nc.sync.dma_start(qtw[:, :, 128:256], qtv[:, :, 128:256])
                    nc.sync.dma_start(ktw[:, :, 128:256], ktv[:, :, 128:256])
            nc.sync.dma_start(
                vtw[:], vt_d[:].rearrange("(t p) m -> p t m", p=128))
            nc.gpsimd.dma_start(
                ot_sb[:], ot_d[:].rearrange("(t p) n -> p t n", p=128))

            nc.vector.memset(ones_f32[:], 1.0)
            for j in range(NB):
                nc.vector.memset(v_sb[j][:, :, 64:65], 1.0)
            make_identity(nc, identf[:])
            make_identity(nc, identb[:])
            # ones row of each ka (static)
            for h in range(HPC):
                nc.gpsimd.dma_start(ka[h][64:65, :], ones_f32[0:16, :])

            negmaxs = {}
            pts = {}

            # ---------------- Phase 1: projections ----------------
            def proj_group_units(p, g):
                # one head-pair, one 512-wide column group: (q,k) chains
                # t-major over the 8 xt f-tiles, then per-head copies on
                # ACT (odd heads base-shift 64->0)
                chains = []
                for w_sb, dstl in ((qtw, qa), (ktw, ka)):
                    ps = pp.tile([128, 512], F32, tag="ps", name="ps")
                    chains.append((ps, w_sb, dstl))

                def tstep(t):
                    for ps, w_sb, dstl in chains:
                        nc.tensor.matmul(
                            ps[:],
                            w_sb[:, t, p * 128:(p + 1) * 128],
                            xt_sb[t][:, g * 512:(g + 1) * 512],
                            start=(t == 0), stop=(t == FT - 1),
                        )

                def copies():
                    cols = slice(g * 512, (g + 1) * 512)
                    for ps, w_sb, dstl in chains:
                        nc.scalar.copy(dstl[2 * p][0:64, cols], ps[0:64, :])
                        nc.scalar.copy(dstl[2 * p + 1][0:64, cols],
                                       ps[64:128, :])

                for t in range(FT):
                    yield (lambda t=t: tstep(t))
                yield copies

            def emit_vproj():
                for sb_i in range(NB):
                    ps = pp.tile([128, 1024], F32, tag="ps", name="vps")
                    for t in range(FT):
                        nc.tensor.matmul(
                            ps[:, 0:256],
                            xt_sb[t][:, sb_i * 128:(sb_i + 1) * 128],
                            vtw[:, t, :],
                            start=(t == 0), stop=(t == FT - 1),
                        )
                    nc.scalar.copy(
                        v_sb[sb_i][:, :, 0:64],
                        ps[:, 0:256].rearrange("p (h d) -> p h d", d=64))

            def A_units(h, pool, tag):
                """Yield per-block emission closures for the A-pass (max)."""
                negmax = ph2.tile([128, NB], F32, tag="negmax",
                                  name=f"negmax{h}", bufs=4)

                def block(i):
                    w = (i + 1) * 128
                    cl = chunks_ge256(w)
                    nch = len(cl)
                    if nch > 1:
                        mp = ph2.tile([128, 4], F32, tag="maxpart",
                                      name=f"mp{h}", bufs=6)
                    off = 0
                    for ci, wc in enumerate(cl):
                        sA = pool.tile([128, 512], F32, tag=tag, name=f"sA{h}")
                        nc.tensor.matmul(
                            sA[:, 0:wc],
                            qa[h][0:64, i * 128:(i + 1) * 128],
                            ka[h][0:64, off:off + wc],
                            start=True, stop=True,
                        )
                        if ci == nch - 1:  # diag: += -BIG*[k>q] via PE
                            dlo = wc - 128
                            nc.tensor.matmul(
                                sA[:, dlo:dlo + 128],
                                rsh_sb[:], tri_sb[:],
                                start=False, stop=True,
                                skip_group_check=True)
                        if nch == 1:
                            nc.vector.reduce_max(
                                negmax[:, i:i + 1], sA[:, 0:wc],
                                axis=mybir.AxisListType.X, negate=True)
                        else:
                            nc.vector.reduce_max(
                                mp[:, ci:ci + 1], sA[:, 0:wc],
                                axis=mybir.AxisListType.X)
                        off += wc
                    if nch > 1:
                        nc.vector.reduce_max(
                            negmax[:, i:i + 1], mp[:, 0:nch],
                            axis=mybir.AxisListType.X, negate=True)

                def aug_dma(g):
                    # negmax cols 4g:4g+4 -> qa[h] row 64, cols g*512:+512.
                    # Split per 4 blocks so T(h)'s early tiles (which need
                    # only low q aug values) unblock before the whole A-pass
                    # has reduced.
                    pst = pool.tile([4, 128], F32, tag=tag, name="pst")
                    nc.tensor.transpose(
                        pst[:], negmax[:, 4 * g:4 * g + 4], identf[:])
                    stage = ph2.tile([4, 128], F32, tag="stage", bufs=8)
                    nc.vector.tensor_copy(stage[:], pst[:])
                    nc.gpsimd.dma_start(
                        qa[h][64:65, g * 512:(g + 1) * 512], stage[:])

                for i in range(NB):
                    yield (lambda i=i: block(i))
                    if i % 4 == 3:
                        yield (lambda g=i // 4: aug_dma(g))

            def vproj_units():
                def block(sb_i):
                    ps = pp.tile([128, 256], F32, tag="ps", name="vps")
                    for t in range(FT):
                        nc.tensor.matmul(
                            ps[:, 0:256],
                            xt_sb[t][:, sb_i * 128:(sb_i + 1) * 128],
                            vtw[:, t, :],
                            start=(t == 0), stop=(t == FT - 1),
                        )
                    nc.scalar.copy(
                        v_sb[sb_i][:, :, 0:64],
                        ps[:, 0:256].rearrange("p (h d) -> p h d", d=64))
                for sb_i in range(NB):
                    yield (lambda sb_i=sb_i: block(sb_i))

            def T_tile(h, j, t0, tile_w):
                tw = min(tile_w, S - t0)
                pt = pts[h]
                sT = psT.tile([128, 1024], F32, tag="sT", name=f"sT{h}")
                coff = 0
                for cw in chunks_aligned(tw):
                    nc.tensor.matmul(
                        sT[:, coff:coff + cw],
                        ka[h][0:65, j * 128:(j + 1) * 128],
                        qa[h][0:65, t0 + coff:t0 + coff + cw],
                        start=True, stop=True,
                    )
                    coff += cw
                if t0 == j * 128:  # diag: += -BIG*[q<k] via PE
                    nc.tensor.matmul(
                        sT[:, 0:128],
                        tri_sb[:], rsh_sb[:],
                        start=False, stop=True,
                        skip_group_check=True)
                nc.scalar.activation(
                    pt[:, PT_OFF[j] + t0 - j * 128:
                       PT_OFF[j] + t0 - j * 128 + tw],
                    sT[:, 0:tw],
                    mybir.ActivationFunctionType.Exp)

            def T_units(h):
                pts[h] = pt_pool.tile([128, PT_COLS], BF16, tag="pt",
                                      name=f"pt{h}")
                for j in range(NB):
                    t0 = j * 128
                    while t0 < S:
                        tw = min(1024, S - t0)
                        yield (lambda j=j, t0=t0: T_tile(h, j, t0, 1024))
                        t0 += tw

            def av_block(h, i, with_ph3):
                pt = pts[h]
                av = psV.tile([128, 65], F32, tag="av", name="av")
                for j in range(i + 1):
                    nc.tensor.matmul(
                        av[:],
                        pt[:, PT_OFF[j] + (i - j) * 128:
                           PT_OFF[j] + (i - j) * 128 + 128],
                        v_sb[j][:, h, :],
                        start=(j == 0), stop=(j == i),
                    )
                recip = ph2.tile([128, 1], F32, tag="recip", bufs=6)
                nc.vector.reciprocal(recip[:], av[:, 64:65])
                nc.vector.tensor_scalar_mul(
                    ho_sb[i][:, h * 64:(h + 1) * 64],
                    av[:, 0:64], recip[:])
                if with_ph3:
                    emit_ph3(i)

            def av_units(h, with_ph3):
                for i in range(NB):
                    yield (lambda i=i: av_block(h, i, with_ph3))

            def run_unit(u):
                if callable(u):
                    u()
                else:
                    u[2]()

            def weave(*streams):
                """Emit units from several streams round-robin by fractional
                progress, preserving order within each stream. Keeps PE fed
                with independent work while another stream's psum slots wait
                on their (slower) DVE consumers."""
                lists = [list(s) for s in streams]
                idx = [0] * len(lists)
                total = sum(len(ls) for ls in lists)
                for _ in range(total):
                    k = min(
                        (j for j in range(len(lists)) if idx[j] < len(lists[j])),
                        key=lambda j: idx[j] / len(lists[j]),
                    )
                    run_unit(lists[k][idx[k]])
                    idx[k] += 1

            def emit_ph3(i):
                hot = ph3.tile([128, 256], BF16, tag="hot", name="hot")
                for t in range(2):
                    ptile = psA.tile([128, 128], BF16, tag="sA", name="ptile")
                    nc.tensor.transpose(
                        ptile[:], ho_sb[i][:, t * 128:(t + 1) * 128], identb[:])
                    # DVE: the tail's ACT is saturated by the last heads'
                    # exp stream while DVE idles there
                    nc.vector.tensor_copy(
                        hot[:, t * 128:(t + 1) * 128], ptile[:])
                ostage = ph3.tile([128, D], BF16, tag="ostage", name="ostage")
                for nchunk in range(2):
                    pot = psA.tile([128, 512], F32, tag="sA", name="pot")
                    for t in range(2):
                        nc.tensor.matmul(
                            pot[:],
                            hot[:, t * 128:(t + 1) * 128],
                            ot_sb[:, t, nchunk * 512:(nchunk + 1) * 512],
                            start=(t == 0), stop=(t == 1),
                        )
                    nc.vector.tensor_copy(
                        ostage[:, nchunk * 512:(nchunk + 1) * 512], pot[:])
                nc.sync.dma_start(out_d[i * 128:(i + 1) * 128, :], ostage[:])

            def braid(h, with_ph3):
                """T(h) and av(h) interleaved per swath: av block j follows
                swath j, so AV/ph3 trail the exp stream block by block
                instead of waiting for the whole T-pass."""
                tu = list(T_units(h))
                au = list(av_units(h, with_ph3))
                out = []
                ti = 0
                for j in range(NB):
                    ntiles = (S - j * 128 + 1023) // 1024
                    out += tu[ti:ti + ntiles]
                    ti += ntiles
                    out.append(au[j])
                return out

            def ladder_units(h, tile_w):
                """Availability-ordered pipeline for one head: A-pass max
                blocks produce aug groups; T tiles are emitted as soon as
                the aug columns they span exist; av blocks follow once
                their swath columns are exp'd. Collapses the per-head tail
                into the A-pass reduce window. Units are tagged triples so
                the tail merger can see the av blocks."""
                au = list(A_units(h, psA, "sA"))   # b0..b3,aug0,b4..,aug3
                pts[h] = pt_pool.tile([128, PT_COLS], BF16, tag="pt",
                                      name=f"pt{h}")
                tiles = []
                tw128 = tile_w // 128
                for j in range(NB):
                    t0 = j * 128
                    while t0 < S:
                        tw = min(tile_w, S - t0)
                        req = j + (t0 - j * 128 + tw + 127) // 128
                        tiles.append(
                            (req, j, lambda j=j, t0=t0:
                             T_tile(h, j, t0, tile_w)))
                        t0 += tw
                tiles.sort(key=lambda x: (x[0], x[1]))
                stream = []
                av_next = 0
                for g in range(4):
                    stream += [("x", 0, u) for u in au[5 * g:5 * g + 5]]
                    rmax = 4 * (g + 1)
                    stream += [("x", 0, u) for (r, j, u) in tiles
                               if 4 * g < r <= rmax]
                    while av_next <= rmax - tw128 and av_next < NB:
                        i = av_next
                        stream.append(
                            ("av", i, lambda i=i: av_block(h, i, False)))
                        av_next += 1
                for i in range(av_next, NB):
                    stream.append(
                        ("av", i, lambda i=i: av_block(h, i, False)))
                return stream

            # phase 1 weave ladder: proj0 groups stream in while earlier
            # groups' A0 max blocks reduce on DVE; then proj1/vproj fill PE
            # while A0/A1/A2 drain. Head h's A-pass finishes as early as
            # its data allows so T(h) can keep the ACT exp stream unbroken.
            A0u = list(A_units(0, psA, "sA"))
            A1u = list(A_units(1, psA, "sA"))
            A2u = list(A_units(2, psA, "sA"))
            G = [list(proj_group_units(0, g)) for g in range(4)]
            for u in G[0]:
                u()
            weave(G[1] + G[2] + G[3], A0u[0:15])
            weave(
                [u for g in range(4) for u in proj_group_units(1, g)],
                A0u[15:20] + A1u[0:10],
            )
            A3u = list(A_units(3, psA, "sA"))
            weave(list(vproj_units()), A1u[10:20] + A2u[0:10])
            ph1_cm.__exit__(None, None, None)   # frees xt/weights SBUF
            pp_cm.__exit__(None, None, None)    # frees 6 PSUM banks

            pt_cm = tc.tile_pool(name="pt_pool", bufs=2)
            ph3_cm = tc.tile_pool(name="ph3", bufs=4)
            psT_cm = tc.tile_pool(name="psT", bufs=2, space="PSUM")
            psV_cm = tc.tile_pool(name="psV", bufs=2, space="PSUM")
            pt_pool, ph3 = pt_cm.__enter__(), ph3_cm.__enter__()
            psT, psV = psT_cm.__enter__(), psV_cm.__enter__()

            # remaining A2 woven through T(0)+av(0); head 3 runs as an
            # availability ladder spread across T(1)/T(2) so its whole
            # chain (incl ph3+stores) finishes inside the DVE reduce window
            # A(h+2)/A(h+3) woven through T(h)+av(h): the max-reduce stream
            # drains on DVE while PE runs the current head's score/AV work
            weave(braid(0, False), A2u[10:20] + A3u[0:10])
            weave(braid(1, False), A3u[10:20])
            # prefetch the first tile of T3's swaths 0..3 into braid(2):
            # their aug groups are ready and ACT has slack there, so exp3
            # starts ~15us earlier and shortens the serial tail
            t3 = list(T_units(3))
            a3 = list(av_units(3, True))
            ntiles3 = [(S - j * 128 + 1023) // 1024 for j in range(NB)]
            starts = []
            ti = 0
            for j in range(NB):
                starts.append(ti)
                ti += ntiles3[j]
            prefetch = [t3[starts[j]] for j in range(4)]
            rest = []
            ti = 0
            for j in range(NB):
                for c in range(ntiles3[j]):
                    if not (j < 4 and c == 0):
                        rest.append(t3[ti])
                    ti += 1
                rest.append(a3[j])
            weave(braid(2, False), prefetch)
            for u in rest:
                u()

            for cm in (psV_cm, psT_cm, ph3_cm, pt_cm, psA_cm, ph2_cm):
                cm.__exit__(None, None, None)

    nc.compile()
    return nc


_NC_CACHE = None


def _get_nc():
    global _NC_CACHE
    if _NC_CACHE is None:
        _NC_CACHE = build_nc()
    return _NC_CACHE


def kernel(x, Q, K, V, O, num_heads=16, _want_results=False, **run_kwargs):
    x = np.asarray(x, dtype=np.float32)
    Q = np.asarray(Q, dtype=np.float32)
    K = np.asarray(K, dtype=np.float32)
    V = np.asarray(V, dtype=np.float32)
    O = np.asarray(O, dtype=np.float32)
    assert x.shape == (B, S, D) and int(num_heads) == H

    idx = np.arange(128)
    # tri[c,k] = [c<=k]; rsh[c,q] = -BIG*[c==q+1]
    # A-side: (rsh.T@tri)[q,k] = -BIG*[k>q]; T-side: (tri.T@rsh)[k,q] = -BIG*[q<k]
    tri = (idx[:, None] <= idx[None, :]).astype(np.float32)
    rsh = np.zeros((128, 128), dtype=np.float32)
    rsh[idx[1:], idx[:-1]] = NEG

    in_maps = []
    for c in range(8):
        b, g = c // 4, c % 4
        rows = slice(g * 256, (g + 1) * 256)
        in_maps.append(dict(
            xt=np.ascontiguousarray(x[b].T),
            qt=np.ascontiguousarray((Q[rows, :] / 8.0).T),
            kt=np.ascontiguousarray(K[rows, :].T),
            vt=np.ascontiguousarray(V[rows, :].T),
            ot=np.ascontiguousarray(O[:, rows].T),
            tri=tri,
            rsh=rsh,
        ))

    nc = _get_nc()
    res = run_bass_kernel_spmd(nc, in_maps, core_ids=list(range(8)), **run_kwargs)

    out = np.zeros((B, S, D), dtype=np.float32)
    for c in range(8):
        out[c // 4] += np.asarray(res.results[c]["out"], dtype=np.float32)
    if _want_results:
        return out, res
    return out


# revision 71
# speedup vs baseline: 1.0206x; 1.0206x over previous
"""Multi-head causal self-attention on 8 TRN2 NeuronCores.

Problem (hardcoded): x[2,2048,1024] f32, Q/K/V/O [1024,1024] f32, 16 heads,
Dh=64, causal softmax, out = attn(x) @ O.T  -> [2,2048,1024] f32.

Sharding: core c handles batch b=c//4 and head group g=c%4 (4 heads each).
Each core computes a partial output (its heads' contribution through the O
projection); the host gather sums the 4 partials per batch (the all-reduce
of the hint, performed at unshard time).

Device algorithm per core (heads h=0..3):
  Phase 1: projections with fp32r matmuls (1 cyc/row at N>=256):
      qa[h][0:64, s] = (Q_h/8) @ x.T   per-head [65, S] tiles; row 64 later
                       receives -rowmax (written by DMA after the A-pass)
      ka[h][0:64, s] = K_h @ x.T       row 64 = ones (set once via DMA)
      v[s,d] bf16 (+ ones column for free softmax denominators)
      PSUM -> SBUF copies ride the DMA engines (frees ACT/DVE).
  Phase 2 per head:
      A-pass: scores[q,k] fp32r -> causal masked row max via DVE (negated)
      negmax [128,16] -PE transpose-> row [1,2048] -DMA-> qa[h] row 64
      T-pass: scores_T[k,q] with K=65 contraction (the ones row of ka times
              the -max row of qa subtracts the row max inside the same
              matmul -- no extra PE cost, cost model charges N only)
              diag causal mask add, then ACT exp -> PT bf16 (k-major)
      av: out[q,d]+denominator via PT.T @ [v|1], normalize by 1/l (DVE)
  Phase 3: ho[s,hd] -PE transpose-> hoT, out_partial = hoT.T @ O_cols.T (bf16)
"""
import numpy as np

import concourse.bass as bass
import concourse.tile as tile
from concourse import bacc, mybir
from concourse.bass_utils import run_bass_kernel_spmd
from concourse.masks import make_identity

F32 = mybir.dt.float32
F32R = mybir.dt.float32r
BF16 = mybir.dt.bfloat16

B, S, D, H = 2, 2048, 1024, 16
DH = 64          # head dim
HPC = 4          # heads per core
NB = S // 128    # 16 q/k blocks
FT = D // 128    # 8 f-tiles
NEG = -3.0e38

# PT column offsets: head-local P^T storage, block j spans q-cols [j*128, S)
PT_OFF = [0] * (NB + 1)
for _j in range(NB):
    PT_OFF[_j + 1] = PT_OFF[_j] + (S - _j * 128)
PT_COLS = PT_OFF[NB]  # 17408


def chunks_ge256(w):
    """Split a multiple-of-128 width into matmul chunks <=512, avoiding
    <256-wide chunks (fp32r moving runs 4 cyc/row below N=256). Only valid
    when each chunk lands in its own PSUM tile (A-pass)."""
    out = []
    while w:
        if w <= 512:
            out.append(w)
            break
        if w == 640:
            out += [384, 256]
            break
        out.append(512)
        w -= 512
    return out


def chunks_aligned(w):
    """512-aligned chunks for matmuls sharing one multi-bank PSUM tile:
    a single matmul output must not cross a PSUM bank (512 f32) boundary."""
    out = []
    while w:
        c = min(512, w)
        out.append(c)
        w -= c
    return out


def build_nc():
    nc = bacc.Bacc(None, target_bir_lowering=False, debug=False)

    # f32r in DRAM: bit-identical to the f32 host arrays, lets the loads go
    # cast-free on the HWDGE (sync) queue in parallel with the Pool queue
    xt_d = nc.dram_tensor("xt", [D, S], F32R, kind="ExternalInput")
    qt_d = nc.dram_tensor("qt", [D, 256], F32R, kind="ExternalInput")
    kt_d = nc.dram_tensor("kt", [D, 256], F32R, kind="ExternalInput")
    vt_d = nc.dram_tensor("vt", [D, 256], F32R, kind="ExternalInput")
    ot_d = nc.dram_tensor("ot", [256, D], F32, kind="ExternalInput")
    tri_d = nc.dram_tensor("tri", [128, 128], F32, kind="ExternalInput")
    rsh_d = nc.dram_tensor("rsh", [128, 128], F32, kind="ExternalInput")
    # bf16 output: halves the store traffic; the host gather sums partials
    # in f32 (bf16 rounding is ~0.4% relative, well inside tolerance)
    out_d = nc.dram_tensor("out", [S, D], BF16, kind="ExternalOutput")

    with tile.TileContext(nc) as tc:
        with (
            tc.tile_pool(name="singles", bufs=1) as singles,
            tc.tile_pool(name="mid", bufs=1) as mid,
        ):
            # whole-kernel constants / tensors
            ot_sb = singles.tile([128, 2, D], BF16)
            v_sb = [singles.tile([128, HPC, 65], BF16, name=f"v{j}")
                    for j in range(NB)]
            ho_sb = [singles.tile([128, 256], BF16, name=f"ho{i}")
                     for i in range(NB)]
            tri_sb = singles.tile([128, 128], BF16)
            rsh_sb = singles.tile([128, 128], BF16)
            ones_f32 = singles.tile([128, 128], F32)
            identf = singles.tile([128, 128], F32)
            identb = singles.tile([128, 128], BF16)

            # per-head augmented projections: rows 0..63 data, row 64 aug
            qa = [mid.tile([65, S], F32R, name=f"qa{h}") for h in range(HPC)]
            ka = [mid.tile([65, S], F32R, name=f"ka{h}") for h in range(HPC)]

            # ---------------- Phase 1: DMAs ----------------
            ph2_cm = tc.tile_pool(name="ph2", bufs=2)
            ph2 = ph2_cm.__enter__()
            psA_cm = tc.tile_pool(name="psA", bufs=2, space="PSUM")
            psA = psA_cm.__enter__()
            ph1_cm = tc.tile_pool(name="ph1", bufs=1)
            pp_cm = tc.tile_pool(name="pp", bufs=6, space="PSUM")
            ph1, pp = ph1_cm.__enter__(), pp_cm.__enter__()

            xt_sb = [ph1.tile([128, S], F32R, name=f"xt_sb{t}")
                     for t in range(FT)]
            qtw = ph1.tile([128, FT, 256], F32R)
            ktw = ph1.tile([128, FT, 256], F32R)
            vtw = ph1.tile([128, FT, 256], F32R)

            # column-group-major xt load: proj0 group g and the A0/A1 max
            # blocks needing cols < (g+1)*512 can run while later groups
            # still stream in; casting loads stay on the Pool queue
            qtv = qt_d[:].rearrange("(t p) m -> p t m", p=128)
            ktv = kt_d[:].rearrange("(t p) m -> p t m", p=128)
            nc.gpsimd.dma_start(tri_sb[:], tri_d[:])
            nc.gpsimd.dma_start(rsh_sb[:], rsh_d[:])
            # only pair-0's weight slices (one strided DMA each) ride
            # ahead of the group-0 xt quarters; pair-1 follows group 1
            nc.sync.dma_start(qtw[:, :, 0:128], qtv[:, :, 0:128])
            nc.sync.dma_start(ktw[:, :, 0:128], ktv[:, :, 0:128])
            for t in range(FT):
                nc.sync.dma_start(
                    xt_sb[t][:, 0:512],
                    xt_d[t * 128:(t + 1) * 128, 0:512])
            for g in range(1, 4):
                for t in range(FT):
                    nc.sync.dma_start(
                        xt_sb[t][:, g * 512:(g + 1) * 512],
                        xt_d[t * 128:(t + 1) * 128, g * 512:(g + 1) * 512])
                if g == 1:
                    nc.sync.dma_start(qtw[:, :, 128:256], qtv[:, :, 128:256])
                    nc.sync.dma_start(ktw[:, :, 128:256], ktv[:, :, 128:256])
            nc.sync.dma_start(
                vtw[:], vt_d[:].rearrange("(t p) m -> p t m", p=128))
            nc.gpsimd.dma_start(
                ot_sb[:], ot_d[:].rearrange("(t p) n -> p t n", p=128))

            nc.vector.memset(ones_f32[:], 1.0)
            for j in range(NB):
                nc.vector.memset(v_sb[j][:, :, 64:65], 1.0)
            make_identity(nc, identf[:])
            make_identity(nc, identb[:])
            # ones row of each ka (static)
            for h in range(HPC):
                nc.gpsimd.dma_start(ka[h][64:65, :], ones_f32[0:16, :])

            negmaxs = {}
            pts = {}

            # ---------------- Phase 1: projections ----------------
            def proj_group_units(p, g):
                # one head-pair, one 512-wide column group: (q,k) chains
                # t-major over the 8 xt f-tiles, then per-head copies on
                # ACT (odd heads base-shift 64->0)
                chains = []
                for w_sb, dstl in ((qtw, qa), (ktw, ka)):
                    ps = pp.tile([128, 512], F32, tag="ps", name="ps")
                    chains.append((ps, w_sb, dstl))

                def tstep(t):
                    for ps, w_sb, dstl in chains:
                        nc.tensor.matmul(
                            ps[:],
                            w_sb[:, t, p * 128:(p + 1) * 128],
                            xt_sb[t][:, g * 512:(g + 1) * 512],
                            start=(t == 0), stop=(t == FT - 1),
                        )

                def copies():
                    cols = slice(g * 512, (g + 1) * 512)
                    for ps, w_sb, dstl in chains:
                        nc.scalar.copy(dstl[2 * p][0:64, cols], ps[0:64, :])
                        nc.scalar.copy(dstl[2 * p + 1][0:64, cols],
                                       ps[64:128, :])

                for t in range(FT):
                    yield (lambda t=t: tstep(t))
                yield copies

            def emit_vproj():
                for sb_i in range(NB):
                    ps = pp.tile([128, 1024], F32, tag="ps", name="vps")
                    for t in range(FT):
                        nc.tensor.matmul(
                            ps[:, 0:256],
                            xt_sb[t][:, sb_i * 128:(sb_i + 1) * 128],
                            vtw[:, t, :],
                            start=(t == 0), stop=(t == FT - 1),
                        )
                    nc.scalar.copy(
                        v_sb[sb_i][:, :, 0:64],
                        ps[:, 0:256].rearrange("p (h d) -> p h d", d=64))

            def A_units(h, pool, tag):
                """Yield per-block emission closures for the A-pass (max)."""
                negmax = ph2.tile([128, NB], F32, tag="negmax",
                                  name=f"negmax{h}", bufs=4)

                def block(i):
                    w = (i + 1) * 128
                    cl = chunks_ge256(w)
                    nch = len(cl)
                    if nch > 1:
                        mp = ph2.tile([128, 4], F32, tag="maxpart",
                                      name=f"mp{h}", bufs=6)
                    off = 0
                    for ci, wc in enumerate(cl):
                        sA = pool.tile([128, 512], F32, tag=tag, name=f"sA{h}")
                        nc.tensor.matmul(
                            sA[:, 0:wc],
                            qa[h][0:64, i * 128:(i + 1) * 128],
                            ka[h][0:64, off:off + wc],
                            start=True, stop=True,
                        )
                        if ci == nch - 1:  # diag: += -BIG*[k>q] via PE
                            dlo = wc - 128
                            nc.tensor.matmul(
                                sA[:, dlo:dlo + 128],
                                rsh_sb[:], tri_sb[:],
                                start=False, stop=True,
                                skip_group_check=True)
                        if nch == 1:
                            nc.vector.reduce_max(
                                negmax[:, i:i + 1], sA[:, 0:wc],
                                axis=mybir.AxisListType.X, negate=True)
                        else:
                            nc.vector.reduce_max(
                                mp[:, ci:ci + 1], sA[:, 0:wc],
                                axis=mybir.AxisListType.X)
                        off += wc
                    if nch > 1:
                        nc.vector.reduce_max(
                            negmax[:, i:i + 1], mp[:, 0:nch],
                            axis=mybir.AxisListType.X, negate=True)

                def aug_dma(g):
                    # negmax cols 4g:4g+4 -> qa[h] row 64, cols g*512:+512.
                    # Split per 4 blocks so T(h)'s early tiles (which need
                    # only low q aug values) unblock before the whole A-pass
                    # has reduced.
                    pst = pool.tile([4, 128], F32, tag=tag, name="pst")
                    nc.tensor.transpose(
                        pst[:], negmax[:, 4 * g:4 * g + 4], identf[:])
                    stage = ph2.tile([4, 128], F32, tag="stage", bufs=8)
                    nc.vector.tensor_copy(stage[:], pst[:])
                    nc.gpsimd.dma_start(
                        qa[h][64:65, g * 512:(g + 1) * 512], stage[:])

                for i in range(NB):
                    yield (lambda i=i: block(i))
                    if i % 4 == 3:
                        yield (lambda g=i // 4: aug_dma(g))

            def vproj_units():
                def block(sb_i):
                    ps = pp.tile([128, 256], F32, tag="ps", name="vps")
                    for t in range(FT):
                        nc.tensor.matmul(
                            ps[:, 0:256],
                            xt_sb[t][:, sb_i * 128:(sb_i + 1) * 128],
                            vtw[:, t, :],
                            start=(t == 0), stop=(t == FT - 1),
                        )
                    nc.scalar.copy(
                        v_sb[sb_i][:, :, 0:64],
                        ps[:, 0:256].rearrange("p (h d) -> p h d", d=64))
                for sb_i in range(NB):
                    yield (lambda sb_i=sb_i: block(sb_i))

            def T_tile(h, j, t0, tile_w):
                tw = min(tile_w, S - t0)
                pt = pts[h]
                sT = psT.tile([128, 1024], F32, tag="sT", name=f"sT{h}")
                coff = 0
                for cw in chunks_aligned(tw):
                    nc.tensor.matmul(
                        sT[:, coff:coff + cw],
                        ka[h][0:65, j * 128:(j + 1) * 128],
                        qa[h][0:65, t0 + coff:t0 + coff + cw],
                        start=True, stop=True,
                    )
                    coff += cw
                if t0 == j * 128:  # diag: += -BIG*[q<k] via PE
                    nc.tensor.matmul(
                        sT[:, 0:128],
                        tri_sb[:], rsh_sb[:],
                        start=False, stop=True,
                        skip_group_check=True)
                nc.scalar.activation(
                    pt[:, PT_OFF[j] + t0 - j * 128:
                       PT_OFF[j] + t0 - j * 128 + tw],
                    sT[:, 0:tw],
                    mybir.ActivationFunctionType.Exp)

            def T_units(h):
                pts[h] = pt_pool.tile([128, PT_COLS], BF16, tag="pt",
                                      name=f"pt{h}")
                for j in range(NB):
                    t0 = j * 128
                    while t0 < S:
                        tw = min(1024, S - t0)
                        yield (lambda j=j, t0=t0: T_tile(h, j, t0, 1024))
                        t0 += tw

            def av_block(h, i, with_ph3):
                pt = pts[h]
                av = psV.tile([128, 65], F32, tag="av", name="av")
                for j in range(i + 1):
                    nc.tensor.matmul(
                        av[:],
                        pt[:, PT_OFF[j] + (i - j) * 128:
                           PT_OFF[j] + (i - j) * 128 + 128],
                        v_sb[j][:, h, :],
                        start=(j == 0), stop=(j == i),
                    )
                recip = ph2.tile([128, 1], F32, tag="recip", bufs=6)
                nc.vector.reciprocal(recip[:], av[:, 64:65])
                nc.vector.tensor_scalar_mul(
                    ho_sb[i][:, h * 64:(h + 1) * 64],
                    av[:, 0:64], recip[:])
                if with_ph3:
                    emit_ph3(i)

            def av_units(h, with_ph3):
                for i in range(NB):
                    yield (lambda i=i: av_block(h, i, with_ph3))

            def run_unit(u):
                if callable(u):
                    u()
                else:
                    u[2]()

            def weave(*streams):
                """Emit units from several streams round-robin by fractional
                progress, preserving order within each stream. Keeps PE fed
                with independent work while another stream's psum slots wait
                on their (slower) DVE consumers."""
                lists = [list(s) for s in streams]
                idx = [0] * len(lists)
                total = sum(len(ls) for ls in lists)
                for _ in range(total):
                    k = min(
                        (j for j in range(len(lists)) if idx[j] < len(lists[j])),
                        key=lambda j: idx[j] / len(lists[j]),
                    )
                    run_unit(lists[k][idx[k]])
                    idx[k] += 1

            def emit_ph3(i):
                hot = ph3.tile([128, 256], BF16, tag="hot", name="hot")
                for t in range(2):
                    ptile = psA.tile([128, 128], BF16, tag="sA", name="ptile")
                    nc.tensor.transpose(
                        ptile[:], ho_sb[i][:, t * 128:(t + 1) * 128], identb[:])
                    # DVE: the tail's ACT is saturated by the last heads'
                    # exp stream while DVE idles there
                    nc.vector.tensor_copy(
                        hot[:, t * 128:(t + 1) * 128], ptile[:])
                ostage = ph3.tile([128, D], BF16, tag="ostage", name="ostage")
                for nchunk in range(2):
                    pot = psA.tile([128, 512], F32, tag="sA", name="pot")
                    for t in range(2):
                        nc.tensor.matmul(
                            pot[:],
                            hot[:, t * 128:(t + 1) * 128],
                            ot_sb[:, t, nchunk * 512:(nchunk + 1) * 512],
                            start=(t == 0), stop=(t == 1),
                        )
                    nc.vector.tensor_copy(
                        ostage[:, nchunk * 512:(nchunk + 1) * 512], pot[:])
                nc.sync.dma_start(out_d[i * 128:(i + 1) * 128, :], ostage[:])

            def braid(h, with_ph3):
                """T(h) and av(h) interleaved per swath: av block j follows
                swath j, so AV/ph3 trail the exp stream block by block
                instead of waiting for the whole T-pass."""
                tu = list(T_units(h))
                au = list(av_units(h, with_ph3))
                out = []
                ti = 0
                for j in range(NB):
                    ntiles = (S - j * 128 + 1023) // 1024
                    out += tu[ti:ti + ntiles]
                    ti += ntiles
                    out.append(au[j])
                return out

            def ladder_units(h, tile_w):
                """Availability-ordered pipeline for one head: A-pass max
                blocks produce aug groups; T tiles are emitted as soon as
                the aug columns they span exist; av blocks follow once
                their swath columns are exp'd. Collapses the per-head tail
                into the A-pass reduce window. Units are tagged triples so
                the tail merger can see the av blocks."""
                au = list(A_units(h, psA, "sA"))   # b0..b3,aug0,b4..,aug3
                pts[h] = pt_pool.tile([128, PT_COLS], BF16, tag="pt",
                                      name=f"pt{h}")
                tiles = []
                tw128 = tile_w // 128
                for j in range(NB):
                    t0 = j * 128
                    while t0 < S:
                        tw = min(tile_w, S - t0)
                        req = j + (t0 - j * 128 + tw + 127) // 128
                        tiles.append(
                            (req, j, lambda j=j, t0=t0:
                             T_tile(h, j, t0, tile_w)))
                        t0 += tw
                tiles.sort(key=lambda x: (x[0], x[1]))
                stream = []
                av_next = 0
                for g in range(4):
                    stream += [("x", 0, u) for u in au[5 * g:5 * g + 5]]
                    rmax = 4 * (g + 1)
                    stream += [("x", 0, u) for (r, j, u) in tiles
                               if 4 * g < r <= rmax]
                    while av_next <= rmax - tw128 and av_next < NB:
                        i = av_next
                        stream.append(
                            ("av", i, lambda i=i: av_block(h, i, False)))
                        av_next += 1
                for i in range(av_next, NB):
                    stream.append(
                        ("av", i, lambda i=i: av_block(h, i, False)))
                return stream

            # phase 1 weave ladder: proj0 groups stream in while earlier
            # groups' A0 max blocks reduce on DVE; then proj1/vproj fill PE
            # while A0/A1/A2 drain. Head h's A-pass finishes as early as
            # its data allows so T(h) can keep the ACT exp stream unbroken.
            A0u = list(A_units(0, psA, "sA"))
            A1u = list(A_units(1, psA, "sA"))
            A2u = list(A_units(2, psA, "sA"))
            G = [list(proj_group_units(0, g)) for g in range(4)]
            for u in G[0]:
                u()
            weave(G[1] + G[2] + G[3], A0u[0:15])
            weave(
                [u for g in range(4) for u in proj_group_units(1, g)],
                A0u[15:20] + A1u[0:10],
            )
            A3u = list(A_units(3, psA, "sA"))
            weave(list(vproj_units()), A1u[10:20] + A2u[0:10])
            ph1_cm.__exit__(None, None, None)   # frees xt/weights SBUF
            pp_cm.__exit__(None, None, None)    # frees 6 PSUM banks

            pt_cm = tc.tile_pool(name="pt_pool", bufs=2)
            ph3_cm = tc.tile_pool(name="ph3", bufs=4)
            psT_cm = tc.tile_pool(name="psT", bufs=2, space="PSUM")
            psV_cm = tc.tile_pool(name="psV", bufs=2, space="PSUM")
            pt_pool, ph3 = pt_cm.__enter__(), ph3_cm.__enter__()
            psT, psV = psT_cm.__enter__(), psV_cm.__enter__()

            # remaining A2 woven through T(0)+av(0); head 3 runs as an
            # availability ladder spread across T(1)/T(2) so its whole
            # chain (incl ph3+stores) finishes inside the DVE reduce window
            # A(h+2)/A(h+3) woven through T(h)+av(h): the max-reduce stream
            # drains on DVE while PE runs the current head's score/AV work
            weave(braid(0, False), A2u[10:20] + A3u[0:12])
            weave(braid(1, False), A3u[12:20])
            for u in braid(2, False):
                u()
            for u in braid(3, True):
                u()

            for cm in (psV_cm, psT_cm, ph3_cm, pt_cm, psA_cm, ph2_cm):
                cm.__exit__(None, None, None)

    nc.compile()
    return nc


_NC_CACHE = None


def _get_nc():
    global _NC_CACHE
    if _NC_CACHE is None:
        _NC_CACHE = build_nc()
    return _NC_CACHE


def kernel(x, Q, K, V, O, num_heads=16, _want_results=False, **run_kwargs):
    x = np.asarray(x, dtype=np.float32)
    Q = np.asarray(Q, dtype=np.float32)
    K = np.asarray(K, dtype=np.float32)
    V = np.asarray(V, dtype=np.float32)
    O = np.asarray(O, dtype=np.float32)
    assert x.shape == (B, S, D) and int(num_heads) == H

    idx = np.arange(128)
    # tri[c,k] = [c<=k]; rsh[c,q] = -BIG*[c==q+1]
    # A-side: (rsh.T@tri)[q,k] = -BIG*[k>q]; T-side: (tri.T@rsh)[k,q] = -BIG*[q<k]
    tri = (idx[:, None] <= idx[None, :]).astype(np.float32)
    rsh = np.zeros((128, 128), dtype=np.float32)
    rsh[idx[1:], idx[:-1]] = NEG

    in_maps = []
    for c in range(8):
        b, g = c // 4, c % 4
        rows = slice(g * 256, (g + 1) * 256)
        in_maps.append(dict(
            xt=np.ascontiguousarray(x[b].T),
            qt=np.ascontiguousarray((Q[rows, :] / 8.0).T),
            kt=np.ascontiguousarray(K[rows, :].T),
            vt=np.ascontiguousarray(V[rows, :].T),
            ot=np.ascontiguousarray(O[:, rows].T),
            tri=tri,
            rsh=rsh,
        ))

    nc = _get_nc()
    res = run_bass_kernel_spmd(nc, in_maps, core_ids=list(range(8)), **run_kwargs)

    out = np.zeros((B, S, D), dtype=np.float32)
    for c in range(8):
        out[c // 4] += np.asarray(res.results[c]["out"], dtype=np.float32)
    if _want_results:
        return out, res
    return out


# revision 72
# speedup vs baseline: 1.0288x; 1.0080x over previous
"""Multi-head causal self-attention on 8 TRN2 NeuronCores.

Problem (hardcoded): x[2,2048,1024] f32, Q/K/V/O [1024,1024] f32, 16 heads,
Dh=64, causal softmax, out = attn(x) @ O.T  -> [2,2048,1024] f32.

Sharding: core c handles batch b=c//4 and head group g=c%4 (4 heads each).
Each core computes a partial output (its heads' contribution through the O
projection); the host gather sums the 4 partials per batch (the all-reduce
of the hint, performed at unshard time).

Device algorithm per core (heads h=0..3):
  Phase 1: projections with fp32r matmuls (1 cyc/row at N>=256):
      qa[h][0:64, s] = (Q_h/8) @ x.T   per-head [65, S] tiles; row 64 later
                       receives -rowmax (written by DMA after the A-pass)
      ka[h][0:64, s] = K_h @ x.T       row 64 = ones (set once via DMA)
      v[s,d] bf16 (+ ones column for free softmax denominators)
      PSUM -> SBUF copies ride the DMA engines (frees ACT/DVE).
  Phase 2 per head:
      A-pass: scores[q,k] fp32r -> causal masked row max via DVE (negated)
      negmax [128,16] -PE transpose-> row [1,2048] -DMA-> qa[h] row 64
      T-pass: scores_T[k,q] with K=65 contraction (the ones row of ka times
              the -max row of qa subtracts the row max inside the same
              matmul -- no extra PE cost, cost model charges N only)
              diag causal mask add, then ACT exp -> PT bf16 (k-major)
      av: out[q,d]+denominator via PT.T @ [v|1], normalize by 1/l (DVE)
  Phase 3: ho[s,hd] -PE transpose-> hoT, out_partial = hoT.T @ O_cols.T (bf16)
"""
import numpy as np

import concourse.bass as bass
import concourse.tile as tile
from concourse import bacc, mybir
from concourse.bass_utils import run_bass_kernel_spmd
from concourse.masks import make_identity

F32 = mybir.dt.float32
F32R = mybir.dt.float32r
BF16 = mybir.dt.bfloat16

B, S, D, H = 2, 2048, 1024, 16
DH = 64          # head dim
HPC = 4          # heads per core
NB = S // 128    # 16 q/k blocks
FT = D // 128    # 8 f-tiles
NEG = -3.0e38

# PT column offsets: head-local P^T storage, block j spans q-cols [j*128, S)
PT_OFF = [0] * (NB + 1)
for _j in range(NB):
    PT_OFF[_j + 1] = PT_OFF[_j] + (S - _j * 128)
PT_COLS = PT_OFF[NB]  # 17408


def chunks_ge256(w):
    """Split a multiple-of-128 width into matmul chunks <=512, avoiding
    <256-wide chunks (fp32r moving runs 4 cyc/row below N=256). Only valid
    when each chunk lands in its own PSUM tile (A-pass)."""
    out = []
    while w:
        if w <= 512:
            out.append(w)
            break
        if w == 640:
            out += [384, 256]
            break
        out.append(512)
        w -= 512
    return out


def chunks_aligned(w):
    """512-aligned chunks for matmuls sharing one multi-bank PSUM tile:
    a single matmul output must not cross a PSUM bank (512 f32) boundary."""
    out = []
    while w:
        c = min(512, w)
        out.append(c)
        w -= c
    return out


def build_nc():
    nc = bacc.Bacc(None, target_bir_lowering=False, debug=False)

    # f32r in DRAM: bit-identical to the f32 host arrays, lets the loads go
    # cast-free on the HWDGE (sync) queue in parallel with the Pool queue
    xt_d = nc.dram_tensor("xt", [D, S], F32R, kind="ExternalInput")
    qt_d = nc.dram_tensor("qt", [D, 256], F32R, kind="ExternalInput")
    kt_d = nc.dram_tensor("kt", [D, 256], F32R, kind="ExternalInput")
    vt_d = nc.dram_tensor("vt", [D, 256], F32R, kind="ExternalInput")
    ot_d = nc.dram_tensor("ot", [256, D], F32, kind="ExternalInput")
    tri_d = nc.dram_tensor("tri", [128, 128], F32, kind="ExternalInput")
    rsh_d = nc.dram_tensor("rsh", [128, 128], F32, kind="ExternalInput")
    # bf16 output: halves the store traffic; the host gather sums partials
    # in f32 (bf16 rounding is ~0.4% relative, well inside tolerance)
    out_d = nc.dram_tensor("out", [S, D], BF16, kind="ExternalOutput")

    with tile.TileContext(nc) as tc:
        with (
            tc.tile_pool(name="singles", bufs=1) as singles,
            tc.tile_pool(name="mid", bufs=1) as mid,
        ):
            # whole-kernel constants / tensors
            ot_sb = singles.tile([128, 2, D], BF16)
            v_sb = [singles.tile([128, HPC, 65], BF16, name=f"v{j}")
                    for j in range(NB)]
            ho_sb = [singles.tile([128, 256], BF16, name=f"ho{i}")
                     for i in range(NB)]
            tri_sb = singles.tile([128, 128], BF16)
            rsh_sb = singles.tile([128, 128], BF16)
            ones_f32 = singles.tile([128, 128], F32)
            identf = singles.tile([128, 128], F32)
            identb = singles.tile([128, 128], BF16)

            # per-head augmented projections: rows 0..63 data, row 64 aug
            qa = [mid.tile([65, S], F32R, name=f"qa{h}") for h in range(HPC)]
            ka = [mid.tile([65, S], F32R, name=f"ka{h}") for h in range(HPC)]

            # ---------------- Phase 1: DMAs ----------------
            ph2_cm = tc.tile_pool(name="ph2", bufs=2)
            ph2 = ph2_cm.__enter__()
            psA_cm = tc.tile_pool(name="psA", bufs=2, space="PSUM")
            psA = psA_cm.__enter__()
            ph1_cm = tc.tile_pool(name="ph1", bufs=1)
            pp_cm = tc.tile_pool(name="pp", bufs=6, space="PSUM")
            ph1, pp = ph1_cm.__enter__(), pp_cm.__enter__()

            xt_sb = [ph1.tile([128, S], F32R, name=f"xt_sb{t}")
                     for t in range(FT)]
            qtw = ph1.tile([128, FT, 256], F32R)
            ktw = ph1.tile([128, FT, 256], F32R)
            vtw = ph1.tile([128, FT, 256], F32R)

            # column-group-major xt load: proj0 group g and the A0/A1 max
            # blocks needing cols < (g+1)*512 can run while later groups
            # still stream in; casting loads stay on the Pool queue
            qtv = qt_d[:].rearrange("(t p) m -> p t m", p=128)
            ktv = kt_d[:].rearrange("(t p) m -> p t m", p=128)
            nc.gpsimd.dma_start(tri_sb[:], tri_d[:])
            nc.gpsimd.dma_start(rsh_sb[:], rsh_d[:])
            # only pair-0's weight slices (one strided DMA each) ride
            # ahead of the group-0 xt quarters; pair-1 follows group 1
            nc.sync.dma_start(qtw[:, :, 0:128], qtv[:, :, 0:128])
            nc.sync.dma_start(ktw[:, :, 0:128], ktv[:, :, 0:128])
            for t in range(FT):
                nc.sync.dma_start(
                    xt_sb[t][:, 0:512],
                    xt_d[t * 128:(t + 1) * 128, 0:512])
            for g in range(1, 4):
                for t in range(FT):
                    nc.sync.dma_start(
                        xt_sb[t][:, g * 512:(g + 1) * 512],
                        xt_d[t * 128:(t + 1) * 128, g * 512:(g + 1) * 512])
                if g == 1:
                    nc.sync.dma_start(qtw[:, :, 128:256], qtv[:, :, 128:256])
                    nc.sync.dma_start(ktw[:, :, 128:256], ktv[:, :, 128:256])
            nc.sync.dma_start(
                vtw[:], vt_d[:].rearrange("(t p) m -> p t m", p=128))
            nc.gpsimd.dma_start(
                ot_sb[:], ot_d[:].rearrange("(t p) n -> p t n", p=128))

            nc.vector.memset(ones_f32[:], 1.0)
            for j in range(NB):
                nc.vector.memset(v_sb[j][:, :, 64:65], 1.0)
            make_identity(nc, identf[:])
            make_identity(nc, identb[:])
            # ones row of each ka (static)
            for h in range(HPC):
                nc.gpsimd.dma_start(ka[h][64:65, :], ones_f32[0:16, :])

            negmaxs = {}
            pts = {}

            # ---------------- Phase 1: projections ----------------
            def proj_group_units(p, g):
                # one head-pair, one 512-wide column group: (q,k) chains
                # t-major over the 8 xt f-tiles, then per-head copies on
                # ACT (odd heads base-shift 64->0)
                chains = []
                for w_sb, dstl in ((qtw, qa), (ktw, ka)):
                    ps = pp.tile([128, 512], F32, tag="ps", name="ps")
                    chains.append((ps, w_sb, dstl))

                def tstep(t):
                    for ps, w_sb, dstl in chains:
                        nc.tensor.matmul(
                            ps[:],
                            w_sb[:, t, p * 128:(p + 1) * 128],
                            xt_sb[t][:, g * 512:(g + 1) * 512],
                            start=(t == 0), stop=(t == FT - 1),
                        )

                def copies():
                    cols = slice(g * 512, (g + 1) * 512)
                    for ps, w_sb, dstl in chains:
                        nc.scalar.copy(dstl[2 * p][0:64, cols], ps[0:64, :])
                        nc.scalar.copy(dstl[2 * p + 1][0:64, cols],
                                       ps[64:128, :])

                for t in range(FT):
                    yield (lambda t=t: tstep(t))
                yield copies

            def emit_vproj():
                for sb_i in range(NB):
                    ps = pp.tile([128, 1024], F32, tag="ps", name="vps")
                    for t in range(FT):
                        nc.tensor.matmul(
                            ps[:, 0:256],
                            xt_sb[t][:, sb_i * 128:(sb_i + 1) * 128],
                            vtw[:, t, :],
                            start=(t == 0), stop=(t == FT - 1),
                        )
                    nc.scalar.copy(
                        v_sb[sb_i][:, :, 0:64],
                        ps[:, 0:256].rearrange("p (h d) -> p h d", d=64))

            def A_units(h, pool, tag):
                """Yield per-block emission closures for the A-pass (max)."""
                negmax = ph2.tile([128, NB], F32, tag="negmax",
                                  name=f"negmax{h}", bufs=4)

                def block(i):
                    w = (i + 1) * 128
                    cl = chunks_ge256(w)
                    nch = len(cl)
                    if nch > 1:
                        mp = ph2.tile([128, 4], F32, tag="maxpart",
                                      name=f"mp{h}", bufs=6)
                    off = 0
                    for ci, wc in enumerate(cl):
                        sA = pool.tile([128, 512], F32, tag=tag, name=f"sA{h}")
                        nc.tensor.matmul(
                            sA[:, 0:wc],
                            qa[h][0:64, i * 128:(i + 1) * 128],
                            ka[h][0:64, off:off + wc],
                            start=True, stop=True,
                        )
                        if ci == nch - 1:  # diag: += -BIG*[k>q] via PE
                            dlo = wc - 128
                            nc.tensor.matmul(
                                sA[:, dlo:dlo + 128],
                                rsh_sb[:], tri_sb[:],
                                start=False, stop=True,
                                skip_group_check=True)
                        if nch == 1:
                            nc.vector.reduce_max(
                                negmax[:, i:i + 1], sA[:, 0:wc],
                                axis=mybir.AxisListType.X, negate=True)
                        else:
                            nc.vector.reduce_max(
                                mp[:, ci:ci + 1], sA[:, 0:wc],
                                axis=mybir.AxisListType.X)
                        off += wc
                    if nch > 1:
                        nc.vector.reduce_max(
                            negmax[:, i:i + 1], mp[:, 0:nch],
                            axis=mybir.AxisListType.X, negate=True)

                def aug_dma(g):
                    # negmax cols 4g:4g+4 -> qa[h] row 64, cols g*512:+512.
                    # Split per 4 blocks so T(h)'s early tiles (which need
                    # only low q aug values) unblock before the whole A-pass
                    # has reduced.
                    pst = pool.tile([4, 128], F32, tag=tag, name="pst")
                    nc.tensor.transpose(
                        pst[:], negmax[:, 4 * g:4 * g + 4], identf[:])
                    stage = ph2.tile([4, 128], F32, tag="stage", bufs=8)
                    nc.vector.tensor_copy(stage[:], pst[:])
                    nc.gpsimd.dma_start(
                        qa[h][64:65, g * 512:(g + 1) * 512], stage[:])

                for i in range(NB):
                    yield (lambda i=i: block(i))
                    if i % 4 == 3:
                        yield (lambda g=i // 4: aug_dma(g))

            def vproj_units():
                def block(sb_i):
                    ps = pp.tile([128, 256], F32, tag="ps", name="vps")
                    for t in range(FT):
                        nc.tensor.matmul(
                            ps[:, 0:256],
                            xt_sb[t][:, sb_i * 128:(sb_i + 1) * 128],
                            vtw[:, t, :],
                            start=(t == 0), stop=(t == FT - 1),
                        )
                    nc.scalar.copy(
                        v_sb[sb_i][:, :, 0:64],
                        ps[:, 0:256].rearrange("p (h d) -> p h d", d=64))
                for sb_i in range(NB):
                    yield (lambda sb_i=sb_i: block(sb_i))

            def T_tile(h, j, t0, tile_w):
                tw = min(tile_w, S - t0)
                pt = pts[h]
                sT = psT.tile([128, 1024], F32, tag="sT", name=f"sT{h}")
                coff = 0
                for cw in chunks_aligned(tw):
                    nc.tensor.matmul(
                        sT[:, coff:coff + cw],
                        ka[h][0:65, j * 128:(j + 1) * 128],
                        qa[h][0:65, t0 + coff:t0 + coff + cw],
                        start=True, stop=True,
                    )
                    coff += cw
                if t0 == j * 128:  # diag: += -BIG*[q<k] via PE
                    nc.tensor.matmul(
                        sT[:, 0:128],
                        tri_sb[:], rsh_sb[:],
                        start=False, stop=True,
                        skip_group_check=True)
                nc.scalar.activation(
                    pt[:, PT_OFF[j] + t0 - j * 128:
                       PT_OFF[j] + t0 - j * 128 + tw],
                    sT[:, 0:tw],
                    mybir.ActivationFunctionType.Exp)

            def T_units(h):
                pts[h] = pt_pool.tile([128, PT_COLS], BF16, tag="pt",
                                      name=f"pt{h}")
                for j in range(NB):
                    t0 = j * 128
                    while t0 < S:
                        tw = min(1024, S - t0)
                        yield (lambda j=j, t0=t0: T_tile(h, j, t0, 1024))
                        t0 += tw

            def av_block(h, i, with_ph3):
                pt = pts[h]
                av = psV.tile([128, 65], F32, tag="av", name="av")
                for j in range(i + 1):
                    nc.tensor.matmul(
                        av[:],
                        pt[:, PT_OFF[j] + (i - j) * 128:
                           PT_OFF[j] + (i - j) * 128 + 128],
                        v_sb[j][:, h, :],
                        start=(j == 0), stop=(j == i),
                    )
                recip = ph2.tile([128, 1], F32, tag="recip", bufs=6)
                nc.vector.reciprocal(recip[:], av[:, 64:65])
                nc.vector.tensor_scalar_mul(
                    ho_sb[i][:, h * 64:(h + 1) * 64],
                    av[:, 0:64], recip[:])
                if with_ph3:
                    emit_ph3(i)

            def av_units(h, with_ph3):
                for i in range(NB):
                    yield (lambda i=i: av_block(h, i, with_ph3))

            def run_unit(u):
                if callable(u):
                    u()
                else:
                    u[2]()

            def weave(*streams):
                """Emit units from several streams round-robin by fractional
                progress, preserving order within each stream. Keeps PE fed
                with independent work while another stream's psum slots wait
                on their (slower) DVE consumers."""
                lists = [list(s) for s in streams]
                idx = [0] * len(lists)
                total = sum(len(ls) for ls in lists)
                for _ in range(total):
                    k = min(
                        (j for j in range(len(lists)) if idx[j] < len(lists[j])),
                        key=lambda j: idx[j] / len(lists[j]),
                    )
                    run_unit(lists[k][idx[k]])
                    idx[k] += 1

            def emit_ph3(i):
                hot = ph3.tile([128, 256], BF16, tag="hot", name="hot")
                for t in range(2):
                    ptile = psA.tile([128, 128], BF16, tag="sA", name="ptile")
                    nc.tensor.transpose(
                        ptile[:], ho_sb[i][:, t * 128:(t + 1) * 128], identb[:])
                    # DVE: the tail's ACT is saturated by the last heads'
                    # exp stream while DVE idles there
                    nc.vector.tensor_copy(
                        hot[:, t * 128:(t + 1) * 128], ptile[:])
                ostage = ph3.tile([128, D], BF16, tag="ostage", name="ostage")
                for nchunk in range(2):
                    pot = psA.tile([128, 512], F32, tag="sA", name="pot")
                    for t in range(2):
                        nc.tensor.matmul(
                            pot[:],
                            hot[:, t * 128:(t + 1) * 128],
                            ot_sb[:, t, nchunk * 512:(nchunk + 1) * 512],
                            start=(t == 0), stop=(t == 1),
                        )
                    nc.vector.tensor_copy(
                        ostage[:, nchunk * 512:(nchunk + 1) * 512], pot[:])
                nc.sync.dma_start(out_d[i * 128:(i + 1) * 128, :], ostage[:])

            def braid(h, with_ph3):
                """T(h) and av(h) interleaved per swath: av block j follows
                swath j, so AV/ph3 trail the exp stream block by block
                instead of waiting for the whole T-pass."""
                tu = list(T_units(h))
                au = list(av_units(h, with_ph3))
                out = []
                ti = 0
                for j in range(NB):
                    ntiles = (S - j * 128 + 1023) // 1024
                    out += tu[ti:ti + ntiles]
                    ti += ntiles
                    out.append(au[j])
                return out

            def ladder_units(h, tile_w):
                """Availability-ordered pipeline for one head: A-pass max
                blocks produce aug groups; T tiles are emitted as soon as
                the aug columns they span exist; av blocks follow once
                their swath columns are exp'd. Collapses the per-head tail
                into the A-pass reduce window. Units are tagged triples so
                the tail merger can see the av blocks."""
                au = list(A_units(h, psA, "sA"))   # b0..b3,aug0,b4..,aug3
                pts[h] = pt_pool.tile([128, PT_COLS], BF16, tag="pt",
                                      name=f"pt{h}")
                tiles = []
                tw128 = tile_w // 128
                for j in range(NB):
                    t0 = j * 128
                    while t0 < S:
                        tw = min(tile_w, S - t0)
                        req = j + (t0 - j * 128 + tw + 127) // 128
                        tiles.append(
                            (req, j, lambda j=j, t0=t0:
                             T_tile(h, j, t0, tile_w)))
                        t0 += tw
                tiles.sort(key=lambda x: (x[0], x[1]))
                stream = []
                av_next = 0
                for g in range(4):
                    stream += [("x", 0, u) for u in au[5 * g:5 * g + 5]]
                    rmax = 4 * (g + 1)
                    stream += [("x", 0, u) for (r, j, u) in tiles
                               if 4 * g < r <= rmax]
                    while av_next <= rmax - tw128 and av_next < NB:
                        i = av_next
                        stream.append(
                            ("av", i, lambda i=i: av_block(h, i, False)))
                        av_next += 1
                for i in range(av_next, NB):
                    stream.append(
                        ("av", i, lambda i=i: av_block(h, i, False)))
                return stream

            # phase 1 weave ladder: proj0 groups stream in while earlier
            # groups' A0 max blocks reduce on DVE; then proj1/vproj fill PE
            # while A0/A1/A2 drain. Head h's A-pass finishes as early as
            # its data allows so T(h) can keep the ACT exp stream unbroken.
            A0u = list(A_units(0, psA, "sA"))
            A1u = list(A_units(1, psA, "sA"))
            A2u = list(A_units(2, psA, "sA"))
            G = [list(proj_group_units(0, g)) for g in range(4)]
            for u in G[0]:
                u()
            weave(G[1] + G[2] + G[3], A0u[0:15])
            weave(
                [u for g in range(4) for u in proj_group_units(1, g)],
                A0u[15:20] + A1u[0:10],
            )
            A3u = list(A_units(3, psA, "sA"))
            weave(list(vproj_units()), A1u[10:20] + A2u[0:10])
            ph1_cm.__exit__(None, None, None)   # frees xt/weights SBUF
            pp_cm.__exit__(None, None, None)    # frees 6 PSUM banks

            pt_cm = tc.tile_pool(name="pt_pool", bufs=2)
            ph3_cm = tc.tile_pool(name="ph3", bufs=4)
            psT_cm = tc.tile_pool(name="psT", bufs=2, space="PSUM")
            psV_cm = tc.tile_pool(name="psV", bufs=2, space="PSUM")
            pt_pool, ph3 = pt_cm.__enter__(), ph3_cm.__enter__()
            psT, psV = psT_cm.__enter__(), psV_cm.__enter__()

            # remaining A2 woven through T(0)+av(0); head 3 runs as an
            # availability ladder spread across T(1)/T(2) so its whole
            # chain (incl ph3+stores) finishes inside the DVE reduce window
            # A(h+2)/A(h+3) woven through T(h)+av(h): the max-reduce stream
            # drains on DVE while PE runs the current head's score/AV work
            weave(braid(0, False), A2u[10:20] + A3u[0:10])
            weave(braid(1, False), A3u[10:20])
            for u in braid(2, False):
                u()
            for u in braid(3, True):
                u()

            for cm in (psV_cm, psT_cm, ph3_cm, pt_cm, psA_cm, ph2_cm):
                cm.__exit__(None, None, None)

    nc.compile()
    return nc


_NC_CACHE = None


def _get_nc():
    global _NC_CACHE
    if _NC_CACHE is None:
        _NC_CACHE = build_nc()
    return _NC_CACHE


def kernel(x, Q, K, V, O, num_heads=16, _want_results=False, **run_kwargs):
    x = np.asarray(x, dtype=np.float32)
    Q = np.asarray(Q, dtype=np.float32)
    K = np.asarray(K, dtype=np.float32)
    V = np.asarray(V, dtype=np.float32)
    O = np.asarray(O, dtype=np.float32)
    assert x.shape == (B, S, D) and int(num_heads) == H

    idx = np.arange(128)
    # tri[c,k] = [c<=k]; rsh[c,q] = -BIG*[c==q+1]
    # A-side: (rsh.T@tri)[q,k] = -BIG*[k>q]; T-side: (tri.T@rsh)[k,q] = -BIG*[q<k]
    tri = (idx[:, None] <= idx[None, :]).astype(np.float32)
    rsh = np.zeros((128, 128), dtype=np.float32)
    rsh[idx[1:], idx[:-1]] = NEG

    in_maps = []
    for c in range(8):
        b, g = c // 4, c % 4
        rows = slice(g * 256, (g + 1) * 256)
        in_maps.append(dict(
            xt=np.ascontiguousarray(x[b].T),
            qt=np.ascontiguousarray((Q[rows, :] / 8.0).T),
            kt=np.ascontiguousarray(K[rows, :].T),
            vt=np.ascontiguousarray(V[rows, :].T),
            ot=np.ascontiguousarray(O[:, rows].T),
            tri=tri,
            rsh=rsh,
        ))

    nc = _get_nc()
    res = run_bass_kernel_spmd(nc, in_maps, core_ids=list(range(8)), **run_kwargs)

    out = np.zeros((B, S, D), dtype=np.float32)
    for c in range(8):
        out[c // 4] += np.asarray(res.results[c]["out"], dtype=np.float32)
    if _want_results:
        return out, res
    return out


# revision 73
# speedup vs baseline: 1.0346x; 1.0057x over previous
"""Multi-head causal self-attention on 8 TRN2 NeuronCores.

Problem (hardcoded): x[2,2048,1024] f32, Q/K/V/O [1024,1024] f32, 16 heads,
Dh=64, causal softmax, out = attn(x) @ O.T  -> [2,2048,1024] f32.

Sharding: core c handles batch b=c//4 and head group g=c%4 (4 heads each).
Each core computes a partial output (its heads' contribution through the O
projection); the host gather sums the 4 partials per batch (the all-reduce
of the hint, performed at unshard time).

Device algorithm per core (heads h=0..3):
  Phase 1: projections with fp32r matmuls (1 cyc/row at N>=256):
      qa[h][0:64, s] = (Q_h/8) @ x.T   per-head [65, S] tiles; row 64 later
                       receives -rowmax (written by DMA after the A-pass)
      ka[h][0:64, s] = K_h @ x.T       row 64 = ones (set once via DMA)
      v[s,d] bf16 (+ ones column for free softmax denominators)
      PSUM -> SBUF copies ride the DMA engines (frees ACT/DVE).
  Phase 2 per head:
      A-pass: scores[q,k] fp32r -> causal masked row max via DVE (negated)
      negmax [128,16] -PE transpose-> row [1,2048] -DMA-> qa[h] row 64
      T-pass: scores_T[k,q] with K=65 contraction (the ones row of ka times
              the -max row of qa subtracts the row max inside the same
              matmul -- no extra PE cost, cost model charges N only)
              diag causal mask add, then ACT exp -> PT bf16 (k-major)
      av: out[q,d]+denominator via PT.T @ [v|1], normalize by 1/l (DVE)
  Phase 3: ho[s,hd] -PE transpose-> hoT, out_partial = hoT.T @ O_cols.T (bf16)
"""
import numpy as np

import concourse.bass as bass
import concourse.tile as tile
from concourse import bacc, mybir
from concourse.bass_utils import run_bass_kernel_spmd
from concourse.masks import make_identity

F32 = mybir.dt.float32
F32R = mybir.dt.float32r
BF16 = mybir.dt.bfloat16

B, S, D, H = 2, 2048, 1024, 16
DH = 64          # head dim
HPC = 4          # heads per core
NB = S // 128    # 16 q/k blocks
FT = D // 128    # 8 f-tiles
NEG = -3.0e38

# PT column offsets: head-local P^T storage, block j spans q-cols [j*128, S)
PT_OFF = [0] * (NB + 1)
for _j in range(NB):
    PT_OFF[_j + 1] = PT_OFF[_j] + (S - _j * 128)
PT_COLS = PT_OFF[NB]  # 17408


def chunks_ge256(w):
    """Split a multiple-of-128 width into matmul chunks <=512, avoiding
    <256-wide chunks (fp32r moving runs 4 cyc/row below N=256). Only valid
    when each chunk lands in its own PSUM tile (A-pass)."""
    out = []
    while w:
        if w <= 512:
            out.append(w)
            break
        if w == 640:
            out += [384, 256]
            break
        out.append(512)
        w -= 512
    return out


def chunks_aligned(w):
    """512-aligned chunks for matmuls sharing one multi-bank PSUM tile:
    a single matmul output must not cross a PSUM bank (512 f32) boundary."""
    out = []
    while w:
        c = min(512, w)
        out.append(c)
        w -= c
    return out


def build_nc():
    nc = bacc.Bacc(None, target_bir_lowering=False, debug=False)

    # f32r in DRAM: bit-identical to the f32 host arrays, lets the loads go
    # cast-free on the HWDGE (sync) queue in parallel with the Pool queue
    xt_d = nc.dram_tensor("xt", [D, S], F32R, kind="ExternalInput")
    qt_d = nc.dram_tensor("qt", [D, 256], F32R, kind="ExternalInput")
    kt_d = nc.dram_tensor("kt", [D, 256], F32R, kind="ExternalInput")
    vt_d = nc.dram_tensor("vt", [D, 256], F32R, kind="ExternalInput")
    ot_d = nc.dram_tensor("ot", [256, D], F32, kind="ExternalInput")
    tri_d = nc.dram_tensor("tri", [128, 128], F32, kind="ExternalInput")
    rsh_d = nc.dram_tensor("rsh", [128, 128], F32, kind="ExternalInput")
    # bf16 output: halves the store traffic; the host gather sums partials
    # in f32 (bf16 rounding is ~0.4% relative, well inside tolerance)
    out_d = nc.dram_tensor("out", [S, D], BF16, kind="ExternalOutput")

    with tile.TileContext(nc) as tc:
        with (
            tc.tile_pool(name="singles", bufs=1) as singles,
            tc.tile_pool(name="mid", bufs=1) as mid,
        ):
            # whole-kernel constants / tensors
            ot_sb = singles.tile([128, 2, D], BF16)
            v_sb = [singles.tile([128, HPC, 65], BF16, name=f"v{j}")
                    for j in range(NB)]
            ho_sb = [singles.tile([128, 256], BF16, name=f"ho{i}")
                     for i in range(NB)]
            tri_sb = singles.tile([128, 128], BF16)
            rsh_sb = singles.tile([128, 128], BF16)
            ones_f32 = singles.tile([128, 128], F32)
            identf = singles.tile([128, 128], F32)
            identb = singles.tile([128, 128], BF16)

            # per-head augmented projections: rows 0..63 data, row 64 aug
            qa = [mid.tile([65, S], F32R, name=f"qa{h}") for h in range(HPC)]
            ka = [mid.tile([65, S], F32R, name=f"ka{h}") for h in range(HPC)]

            # ---------------- Phase 1: DMAs ----------------
            ph2_cm = tc.tile_pool(name="ph2", bufs=2)
            ph2 = ph2_cm.__enter__()
            psA_cm = tc.tile_pool(name="psA", bufs=2, space="PSUM")
            psA = psA_cm.__enter__()
            ph1_cm = tc.tile_pool(name="ph1", bufs=1)
            pp_cm = tc.tile_pool(name="pp", bufs=6, space="PSUM")
            ph1, pp = ph1_cm.__enter__(), pp_cm.__enter__()

            xt_sb = [ph1.tile([128, S], F32R, name=f"xt_sb{t}")
                     for t in range(FT)]
            qtw = ph1.tile([128, FT, 256], F32R)
            ktw = ph1.tile([128, FT, 256], F32R)
            vtw = ph1.tile([128, FT, 256], F32R)

            # column-group-major xt load: proj0 group g and the A0/A1 max
            # blocks needing cols < (g+1)*512 can run while later groups
            # still stream in; casting loads stay on the Pool queue
            qtv = qt_d[:].rearrange("(t p) m -> p t m", p=128)
            ktv = kt_d[:].rearrange("(t p) m -> p t m", p=128)
            nc.gpsimd.dma_start(tri_sb[:], tri_d[:])
            nc.gpsimd.dma_start(rsh_sb[:], rsh_d[:])
            # only pair-0's weight slices (one strided DMA each) ride
            # ahead of the group-0 xt quarters; pair-1 follows group 1
            nc.sync.dma_start(qtw[:, :, 0:128], qtv[:, :, 0:128])
            nc.sync.dma_start(ktw[:, :, 0:128], ktv[:, :, 0:128])
            for t in range(FT):
                nc.sync.dma_start(
                    xt_sb[t][:, 0:512],
                    xt_d[t * 128:(t + 1) * 128, 0:512])
            for g in range(1, 4):
                for t in range(FT):
                    nc.sync.dma_start(
                        xt_sb[t][:, g * 512:(g + 1) * 512],
                        xt_d[t * 128:(t + 1) * 128, g * 512:(g + 1) * 512])
                if g == 1:
                    nc.sync.dma_start(qtw[:, :, 128:256], qtv[:, :, 128:256])
                    nc.sync.dma_start(ktw[:, :, 128:256], ktv[:, :, 128:256])
            nc.sync.dma_start(
                vtw[:], vt_d[:].rearrange("(t p) m -> p t m", p=128))
            nc.gpsimd.dma_start(
                ot_sb[:], ot_d[:].rearrange("(t p) n -> p t n", p=128))

            nc.vector.memset(ones_f32[:], 1.0)
            for j in range(NB):
                nc.vector.memset(v_sb[j][:, :, 64:65], 1.0)
            make_identity(nc, identf[:])
            make_identity(nc, identb[:])
            # ones row of each ka (static)
            for h in range(HPC):
                nc.gpsimd.dma_start(ka[h][64:65, :], ones_f32[0:16, :])

            negmaxs = {}
            pts = {}

            # ---------------- Phase 1: projections ----------------
            def proj_group_units(p, g):
                # one head-pair, one 512-wide column group: (q,k) chains
                # t-major over the 8 xt f-tiles, then per-head copies on
                # ACT (odd heads base-shift 64->0)
                chains = []
                for w_sb, dstl in ((qtw, qa), (ktw, ka)):
                    ps = pp.tile([128, 512], F32, tag="ps", name="ps")
                    chains.append((ps, w_sb, dstl))

                def tstep(t):
                    for ps, w_sb, dstl in chains:
                        nc.tensor.matmul(
                            ps[:],
                            w_sb[:, t, p * 128:(p + 1) * 128],
                            xt_sb[t][:, g * 512:(g + 1) * 512],
                            start=(t == 0), stop=(t == FT - 1),
                        )

                def copies():
                    cols = slice(g * 512, (g + 1) * 512)
                    for ps, w_sb, dstl in chains:
                        nc.scalar.copy(dstl[2 * p][0:64, cols], ps[0:64, :])
                        nc.scalar.copy(dstl[2 * p + 1][0:64, cols],
                                       ps[64:128, :])

                for t in range(FT):
                    yield (lambda t=t: tstep(t))
                yield copies

            def emit_vproj():
                for sb_i in range(NB):
                    ps = pp.tile([128, 1024], F32, tag="ps", name="vps")
                    for t in range(FT):
                        nc.tensor.matmul(
                            ps[:, 0:256],
                            xt_sb[t][:, sb_i * 128:(sb_i + 1) * 128],
                            vtw[:, t, :],
                            start=(t == 0), stop=(t == FT - 1),
                        )
                    nc.scalar.copy(
                        v_sb[sb_i][:, :, 0:64],
                        ps[:, 0:256].rearrange("p (h d) -> p h d", d=64))

            def A_units(h, pool, tag):
                """Yield per-block emission closures for the A-pass (max)."""
                negmax = ph2.tile([128, NB], F32, tag="negmax",
                                  name=f"negmax{h}", bufs=4)

                def block(i):
                    w = (i + 1) * 128
                    cl = chunks_ge256(w)
                    nch = len(cl)
                    if nch > 1:
                        mp = ph2.tile([128, 4], F32, tag="maxpart",
                                      name=f"mp{h}", bufs=6)
                    off = 0
                    for ci, wc in enumerate(cl):
                        sA = pool.tile([128, 512], F32, tag=tag, name=f"sA{h}")
                        nc.tensor.matmul(
                            sA[:, 0:wc],
                            qa[h][0:64, i * 128:(i + 1) * 128],
                            ka[h][0:64, off:off + wc],
                            start=True, stop=True,
                        )
                        if ci == nch - 1:  # diag: += -BIG*[k>q] via PE
                            dlo = wc - 128
                            nc.tensor.matmul(
                                sA[:, dlo:dlo + 128],
                                rsh_sb[:], tri_sb[:],
                                start=False, stop=True,
                                skip_group_check=True)
                        if nch == 1:
                            nc.vector.reduce_max(
                                negmax[:, i:i + 1], sA[:, 0:wc],
                                axis=mybir.AxisListType.X, negate=True)
                        else:
                            nc.vector.reduce_max(
                                mp[:, ci:ci + 1], sA[:, 0:wc],
                                axis=mybir.AxisListType.X)
                        off += wc
                    if nch > 1:
                        nc.vector.reduce_max(
                            negmax[:, i:i + 1], mp[:, 0:nch],
                            axis=mybir.AxisListType.X, negate=True)

                def aug_dma(g):
                    # negmax cols 4g:4g+4 -> qa[h] row 64, cols g*512:+512.
                    # Split per 4 blocks so T(h)'s early tiles (which need
                    # only low q aug values) unblock before the whole A-pass
                    # has reduced.
                    pst = pool.tile([4, 128], F32, tag=tag, name="pst")
                    nc.tensor.transpose(
                        pst[:], negmax[:, 4 * g:4 * g + 4], identf[:])
                    stage = ph2.tile([4, 128], F32, tag="stage", bufs=8)
                    nc.vector.tensor_copy(stage[:], pst[:])
                    nc.gpsimd.dma_start(
                        qa[h][64:65, g * 512:(g + 1) * 512], stage[:])

                for i in range(NB):
                    yield (lambda i=i: block(i))
                    if i % 4 == 3:
                        yield (lambda g=i // 4: aug_dma(g))

            def vproj_units():
                def block(sb_i):
                    ps = pp.tile([128, 256], F32, tag="ps", name="vps")
                    for t in range(FT):
                        nc.tensor.matmul(
                            ps[:, 0:256],
                            xt_sb[t][:, sb_i * 128:(sb_i + 1) * 128],
                            vtw[:, t, :],
                            start=(t == 0), stop=(t == FT - 1),
                        )
                    nc.scalar.copy(
                        v_sb[sb_i][:, :, 0:64],
                        ps[:, 0:256].rearrange("p (h d) -> p h d", d=64))
                for sb_i in range(NB):
                    yield (lambda sb_i=sb_i: block(sb_i))

            def T_tile(h, j, t0, tile_w):
                tw = min(tile_w, S - t0)
                pt = pts[h]
                sT = psT.tile([128, 1024], F32, tag="sT", name=f"sT{h}")
                coff = 0
                for cw in chunks_aligned(tw):
                    nc.tensor.matmul(
                        sT[:, coff:coff + cw],
                        ka[h][0:65, j * 128:(j + 1) * 128],
                        qa[h][0:65, t0 + coff:t0 + coff + cw],
                        start=True, stop=True,
                    )
                    coff += cw
                if t0 == j * 128:  # diag: += -BIG*[q<k] via PE
                    nc.tensor.matmul(
                        sT[:, 0:128],
                        tri_sb[:], rsh_sb[:],
                        start=False, stop=True,
                        skip_group_check=True)
                nc.scalar.activation(
                    pt[:, PT_OFF[j] + t0 - j * 128:
                       PT_OFF[j] + t0 - j * 128 + tw],
                    sT[:, 0:tw],
                    mybir.ActivationFunctionType.Exp)

            def T_units(h):
                pts[h] = pt_pool.tile([128, PT_COLS], BF16, tag="pt",
                                      name=f"pt{h}")
                for j in range(NB):
                    t0 = j * 128
                    while t0 < S:
                        tw = min(1024, S - t0)
                        yield (lambda j=j, t0=t0: T_tile(h, j, t0, 1024))
                        t0 += tw

            def av_block(h, i, with_ph3):
                pt = pts[h]
                av = psV.tile([128, 65], F32, tag="av", name="av")
                for j in range(i + 1):
                    nc.tensor.matmul(
                        av[:],
                        pt[:, PT_OFF[j] + (i - j) * 128:
                           PT_OFF[j] + (i - j) * 128 + 128],
                        v_sb[j][:, h, :],
                        start=(j == 0), stop=(j == i),
                    )
                recip = ph2.tile([128, 1], F32, tag="recip", bufs=6)
                nc.vector.reciprocal(recip[:], av[:, 64:65])
                nc.vector.tensor_scalar_mul(
                    ho_sb[i][:, h * 64:(h + 1) * 64],
                    av[:, 0:64], recip[:])
                if with_ph3:
                    emit_ph3(i)

            def av_units(h, with_ph3):
                for i in range(NB):
                    yield (lambda i=i: av_block(h, i, with_ph3))

            def run_unit(u):
                if callable(u):
                    u()
                else:
                    u[2]()

            def weave(*streams):
                """Emit units from several streams round-robin by fractional
                progress, preserving order within each stream. Keeps PE fed
                with independent work while another stream's psum slots wait
                on their (slower) DVE consumers."""
                lists = [list(s) for s in streams]
                idx = [0] * len(lists)
                total = sum(len(ls) for ls in lists)
                for _ in range(total):
                    k = min(
                        (j for j in range(len(lists)) if idx[j] < len(lists[j])),
                        key=lambda j: idx[j] / len(lists[j]),
                    )
                    run_unit(lists[k][idx[k]])
                    idx[k] += 1

            def emit_ph3(i):
                hot = ph3.tile([128, 256], BF16, tag="hot", name="hot")
                for t in range(2):
                    ptile = psA.tile([128, 128], BF16, tag="sA", name="ptile")
                    nc.tensor.transpose(
                        ptile[:], ho_sb[i][:, t * 128:(t + 1) * 128], identb[:])
                    # DVE: the tail's ACT is saturated by the last heads'
                    # exp stream while DVE idles there
                    nc.vector.tensor_copy(
                        hot[:, t * 128:(t + 1) * 128], ptile[:])
                ostage = ph3.tile([128, D], BF16, tag="ostage", name="ostage")
                for nchunk in range(2):
                    pot = psA.tile([128, 512], F32, tag="sA", name="pot")
                    for t in range(2):
                        nc.tensor.matmul(
                            pot[:],
                            hot[:, t * 128:(t + 1) * 128],
                            ot_sb[:, t, nchunk * 512:(nchunk + 1) * 512],
                            start=(t == 0), stop=(t == 1),
                        )
                    nc.vector.tensor_copy(
                        ostage[:, nchunk * 512:(nchunk + 1) * 512], pot[:])
                nc.sync.dma_start(out_d[i * 128:(i + 1) * 128, :], ostage[:])

            def braid(h, with_ph3):
                """T(h) and av(h) interleaved per swath: av block j follows
                swath j, so AV/ph3 trail the exp stream block by block
                instead of waiting for the whole T-pass."""
                tu = list(T_units(h))
                au = list(av_units(h, with_ph3))
                out = []
                ti = 0
                # av blocks lag one swath behind the T tiles: PE then always
                # has the next swath's matmuls queued while ACT exps the
                # previous one, instead of ping-ponging per swath
                for j in range(NB):
                    ntiles = (S - j * 128 + 1023) // 1024
                    out += tu[ti:ti + ntiles]
                    ti += ntiles
                    if j >= 1:
                        out.append(au[j - 1])
                out.append(au[NB - 1])
                return out

            def ladder_units(h, tile_w):
                """Availability-ordered pipeline for one head: A-pass max
                blocks produce aug groups; T tiles are emitted as soon as
                the aug columns they span exist; av blocks follow once
                their swath columns are exp'd. Collapses the per-head tail
                into the A-pass reduce window. Units are tagged triples so
                the tail merger can see the av blocks."""
                au = list(A_units(h, psA, "sA"))   # b0..b3,aug0,b4..,aug3
                pts[h] = pt_pool.tile([128, PT_COLS], BF16, tag="pt",
                                      name=f"pt{h}")
                tiles = []
                tw128 = tile_w // 128
                for j in range(NB):
                    t0 = j * 128
                    while t0 < S:
                        tw = min(tile_w, S - t0)
                        req = j + (t0 - j * 128 + tw + 127) // 128
                        tiles.append(
                            (req, j, lambda j=j, t0=t0:
                             T_tile(h, j, t0, tile_w)))
                        t0 += tw
                tiles.sort(key=lambda x: (x[0], x[1]))
                stream = []
                av_next = 0
                for g in range(4):
                    stream += [("x", 0, u) for u in au[5 * g:5 * g + 5]]
                    rmax = 4 * (g + 1)
                    stream += [("x", 0, u) for (r, j, u) in tiles
                               if 4 * g < r <= rmax]
                    while av_next <= rmax - tw128 and av_next < NB:
                        i = av_next
                        stream.append(
                            ("av", i, lambda i=i: av_block(h, i, False)))
                        av_next += 1
                for i in range(av_next, NB):
                    stream.append(
                        ("av", i, lambda i=i: av_block(h, i, False)))
                return stream

            # phase 1 weave ladder: proj0 groups stream in while earlier
            # groups' A0 max blocks reduce on DVE; then proj1/vproj fill PE
            # while A0/A1/A2 drain. Head h's A-pass finishes as early as
            # its data allows so T(h) can keep the ACT exp stream unbroken.
            A0u = list(A_units(0, psA, "sA"))
            A1u = list(A_units(1, psA, "sA"))
            A2u = list(A_units(2, psA, "sA"))
            G = [list(proj_group_units(0, g)) for g in range(4)]
            for u in G[0]:
                u()
            weave(G[1] + G[2] + G[3], A0u[0:15])
            weave(
                [u for g in range(4) for u in proj_group_units(1, g)],
                A0u[15:20] + A1u[0:10],
            )
            A3u = list(A_units(3, psA, "sA"))
            weave(list(vproj_units()), A1u[10:20] + A2u[0:10])
            ph1_cm.__exit__(None, None, None)   # frees xt/weights SBUF
            pp_cm.__exit__(None, None, None)    # frees 6 PSUM banks

            pt_cm = tc.tile_pool(name="pt_pool", bufs=2)
            ph3_cm = tc.tile_pool(name="ph3", bufs=4)
            psT_cm = tc.tile_pool(name="psT", bufs=2, space="PSUM")
            psV_cm = tc.tile_pool(name="psV", bufs=2, space="PSUM")
            pt_pool, ph3 = pt_cm.__enter__(), ph3_cm.__enter__()
            psT, psV = psT_cm.__enter__(), psV_cm.__enter__()

            # remaining A2 woven through T(0)+av(0); head 3 runs as an
            # availability ladder spread across T(1)/T(2) so its whole
            # chain (incl ph3+stores) finishes inside the DVE reduce window
            # A(h+2)/A(h+3) woven through T(h)+av(h): the max-reduce stream
            # drains on DVE while PE runs the current head's score/AV work
            weave(braid(0, False), A2u[10:20] + A3u[0:10])
            weave(braid(1, False), A3u[10:20])
            for u in braid(2, False):
                u()
            for u in braid(3, True):
                u()

            for cm in (psV_cm, psT_cm, ph3_cm, pt_cm, psA_cm, ph2_cm):
                cm.__exit__(None, None, None)

    nc.compile()
    return nc


_NC_CACHE = None


def _get_nc():
    global _NC_CACHE
    if _NC_CACHE is None:
        _NC_CACHE = build_nc()
    return _NC_CACHE


def kernel(x, Q, K, V, O, num_heads=16, _want_results=False, **run_kwargs):
    x = np.asarray(x, dtype=np.float32)
    Q = np.asarray(Q, dtype=np.float32)
    K = np.asarray(K, dtype=np.float32)
    V = np.asarray(V, dtype=np.float32)
    O = np.asarray(O, dtype=np.float32)
    assert x.shape == (B, S, D) and int(num_heads) == H

    idx = np.arange(128)
    # tri[c,k] = [c<=k]; rsh[c,q] = -BIG*[c==q+1]
    # A-side: (rsh.T@tri)[q,k] = -BIG*[k>q]; T-side: (tri.T@rsh)[k,q] = -BIG*[q<k]
    tri = (idx[:, None] <= idx[None, :]).astype(np.float32)
    rsh = np.zeros((128, 128), dtype=np.float32)
    rsh[idx[1:], idx[:-1]] = NEG

    in_maps = []
    for c in range(8):
        b, g = c // 4, c % 4
        rows = slice(g * 256, (g + 1) * 256)
        in_maps.append(dict(
            xt=np.ascontiguousarray(x[b].T),
            qt=np.ascontiguousarray((Q[rows, :] / 8.0).T),
            kt=np.ascontiguousarray(K[rows, :].T),
            vt=np.ascontiguousarray(V[rows, :].T),
            ot=np.ascontiguousarray(O[:, rows].T),
            tri=tri,
            rsh=rsh,
        ))

    nc = _get_nc()
    res = run_bass_kernel_spmd(nc, in_maps, core_ids=list(range(8)), **run_kwargs)

    out = np.zeros((B, S, D), dtype=np.float32)
    for c in range(8):
        out[c // 4] += np.asarray(res.results[c]["out"], dtype=np.float32)
    if _want_results:
        return out, res
    return out
